# revision 3
# baseline (speedup 1.0000x reference)
"""Causal self-attention (B=4, T=2048, C=1024, H=16, hd=64) on 8 trn2 cores.

Sharding: core c -> batch b = c//2, head-half hh = c%2 (8 heads each);
host sums the two partial c_proj outputs per batch.

fp8(e4m3) DoubleRow design (0.5 cyc/row on PE):
  host prep: xT hi/lo fp8, wqk*32 hi/lo fp8, wv*32 hi/lo fp8, wp/32 bf16
             (head-pair packed). Scales keep all fp8 values in normal range.
  phase1: qkT = 3-chain hi/lo fp8 DoubleRow (xh*wh + xh*wl + xl*wh ~ 0.1% err)
          evac: q8 (e4m3), k hi/lo pair (exact-ish), V 3-chain -> v8 e4m3;
          tokens<512 also evac'd to bf16 (qb/kb/vb) for the exact qc0 path.
  qc0 (rows 0-511): exact bf16 attention (small-n softmax rows are the
          error tail under fp8; bf16 keeps them ~4e-3).
  qc1-3: S^T = (kh|kl)-DoubleRow @ broadcast q8 (k exact-ish, q single fp8);
          exp on ACT -> P e4m3 (bias ln8 keeps P in normal range);
          causal via gpsimd affine_select bands; PV + den DoubleRow over
          kt-pairs (den separate M=1 matmul: M=65 stationary fails ISA).
  proj:   yT2 bf16 head-pair packed (K=128) @ wp bf16, per finished q-chunk.
"""

import numpy as np
import ml_dtypes

import concourse.bass as bass
import concourse.mybir as mybir
import concourse.tile as tile
from concourse.bass_utils import run_bass_kernel_spmd

F32 = mybir.dt.float32
BF16 = mybir.dt.bfloat16
F8 = mybir.dt.float8e4
EXP = mybir.ActivationFunctionType.Exp
DR = mybir.MatmulPerfMode.DoubleRow
F8NP = ml_dtypes.float8_e4m3
BFNP = ml_dtypes.bfloat16

B = 4
T = 2048
C = 1024
HD = 64
NHL = 8            # heads per core
TCH = 256          # phase-1 token chunk
QC = 512           # q chunk width
SW = 32.0          # fp8 weight/activation scale
SCALE = 0.125 / (SW * SW)   # exp scale on the 32*q . 32*k psum
PBIAS = float(np.log(8.0))  # P stored as 8*exp(S): e4m3 normal range


def _build_nc():
    nc = bass.Bass("TRN2", target_bir_lowering=False, debug=False)

    xh_d = nc.dram_tensor("xh", [C, T], F8, kind="ExternalInput")
    xl_d = nc.dram_tensor("xl", [C, T], F8, kind="ExternalInput")
    wqkh_d = nc.dram_tensor("wqkh", [C, 1024], F8, kind="ExternalInput")
    wqkl_d = nc.dram_tensor("wqkl", [C, 1024], F8, kind="ExternalInput")
    wvh_d = nc.dram_tensor("wvh", [C, 512], F8, kind="ExternalInput")
    wvl_d = nc.dram_tensor("wvl", [C, 512], F8, kind="ExternalInput")
    wp_d = nc.dram_tensor("wp", [128, 4, 1024], BF16, kind="ExternalInput")
    out_d = nc.dram_tensor("out", [T, C], F32, kind="ExternalOutput")

    with tile.TileContext(nc) as tc:
        _emit(tc, xh_d, xl_d, wqkh_d, wqkl_d, wvh_d, wvl_d, wp_d, out_d.ap())
    _split_multi_waits(nc)
    return nc


def _split_multi_waits(nc):
    """Walrus accepts only one sync-wait per PE-queue instruction; hoist
    extra waits onto same-engine NoOps inserted right before."""
    nid = [0]
    for f in nc.m.functions:
        for blk in f.blocks:
            out = []
            changed = False
            for inst in blk.instructions:
                si = inst.sync_info
                if si is not None and len(si.on_wait) > 1:
                    waits = list(si.on_wait)
                    for w in waits[:-1]:
                        nop = mybir.InstNoOp(name=f"I-waitnop-{nid[0]}")
                        nid[0] += 1
                        nop.engine = inst.engine
                        nop.sync_info = mybir.SyncInfo(on_wait=[w], on_update=[])
                        out.append(nop)
                    inst.sync_info = mybir.SyncInfo(
                        on_wait=[waits[-1]], on_update=list(si.on_update)
                    )
                    changed = True
                out.append(inst)
            if changed:
                blk.instructions = out


def _emit(tc, xh_d, xl_d, wqkh_d, wqkl_d, wvh_d, wvl_d, wp_d, out_d):
    nc = tc.nc
    r8 = lambda d: d.ap().rearrange("(cc p) f -> p cc f", p=128)

    with tc.tile_pool(name="persist", bufs=1) as pp, \
         tc.tile_pool(name="p1x", bufs=1) as p1x, \
         tc.tile_pool(name="p2", bufs=4) as p2, \
         tc.tile_pool(name="p2y", bufs=4) as p2y, \
         tc.tile_pool(name="p2r", bufs=3) as p2r, \
         tc.tile_pool(name="p3", bufs=3) as p3, \
         tc.tile_pool(name="drp", bufs=8, space="DRAM") as drp, \
         tc.tile_pool(name="ps_s", bufs=2, space="PSUM") as ps_s, \
         tc.tile_pool(name="ps_o", bufs=2, space="PSUM") as ps_o, \
         tc.tile_pool(name="ps_a", bufs=2, space="PSUM") as ps_a:
        # ---- persistent SBUF ----
        wqkh = pp.tile([128, 8, 1024], F8)
        wqkl = pp.tile([128, 8, 1024], F8)
        wvh = pp.tile([128, 8, 512], F8)
        wvl = pp.tile([128, 8, 512], F8)
        wp = pp.tile([128, 4, 1024], BF16)

        q8 = pp.tile([128, 4, T], F8)          # [pair-part, hp, tok]
        khl = pp.tile([128, 4, 2, T], F8)      # [pair-part, hp, hi/lo, tok]
        qb = pp.tile([128, 4, QC], BF16)       # tokens<512 bf16
        kb = pp.tile([128, 4, QC], BF16)
        v8 = pp.tile([128, 16, 8, 128], F8)    # [tok-part, kt, h, v|ones|junk]
        vb = pp.tile([128, 4, 8 * 65], BF16)   # kt<4, [.., h*65+d | ones]
        ebias = pp.tile([128, 1], F32)
        onesr = pp.tile([1, 64], F32)
        nc.vector.memset(onesr[:], 1.0)
        nc.gpsimd.memset(v8[:], 0.0)
        nc.gpsimd.memset(v8[:, :, :, 64:65], 1.0)
        nc.vector.memset(ebias[:], PBIAS)
        vob = vb[:].rearrange("p kt (l c) -> p kt l c", c=65)[:, :, :, 64:65]
        nc.vector.memset(vob, 1.0)

        xtiles = {}

        def phase1_j(ch, j):
            xh, xl = xtiles[ch]
            ts = bass.ds(ch * TCH, TCH)
            psq_t = ps_a.tile([128, 512], F32, tag="pa")
            psq = psq_t[:, 0:TCH]
            chains = ([(wqkh, xh)] if (j < 4 and ch >= 2)
                      else [(wqkh, xh), (wqkl, xh), (wqkh, xl)])
            nlast = 4 * len(chains) - 1
            n = 0
            for w, xx in chains:
                for p in range(4):
                    nc.tensor.matmul(
                        psq,
                        w[:, 2 * p:2 * p + 2, j * 128:(j + 1) * 128],
                        xx[:, 2 * p:2 * p + 2, :],
                        start=(n == 0), stop=(n == nlast), perf_mode=DR)
                    n += 1
            if j < 4:
                nc.vector.tensor_copy(q8[:, j, ts], psq)
                if ch < 2:
                    nc.vector.tensor_copy(qb[:, j, ts], psq)
            else:
                hp = j - 4
                nc.vector.tensor_copy(khl[:, hp, 0, ts], psq)
                nc.vector.scalar_tensor_tensor(
                    khl[:, hp, 1, ts], psq, 1.0, khl[:, hp, 0, ts],
                    op0=mybir.AluOpType.mult, op1=mybir.AluOpType.subtract)
                if ch < 2:
                    nc.vector.tensor_copy(kb[:, hp, ts], psq)

        def phase1_v(ch, s):
            xh, xl = xtiles[ch]
            psv = ps_a.tile([128, 512], F32, tag="pa")
            chains = ([(wvh, xh)] if ch >= 2
                      else [(wvh, xh), (wvl, xh), (wvh, xl)])
            nlast = 4 * len(chains) - 1
            n = 0
            for w, xx in chains:
                for p in range(4):
                    nc.tensor.matmul(
                        psv[:],
                        xx[:, 2 * p:2 * p + 2, s * 128:(s + 1) * 128],
                        w[:, 2 * p:2 * p + 2, :],
                        start=(n == 0), stop=(n == nlast), perf_mode=DR)
                    n += 1
            kt = 2 * ch + s
            nc.vector.tensor_copy(
                v8[:, kt, :, 0:64],
                psv[:].rearrange("p (l c) -> p l c", c=64))
            if ch < 2:
                nc.vector.tensor_copy(
                    vb[:, kt, :].rearrange("p (l c) -> p l c", c=65)[:, :, 0:64],
                    psv[:].rearrange("p (l c) -> p l c", c=64))

        def phase1_units(ch):
            yield lambda: phase1_j(ch, 0)
            for jj in range(1, 8):
                yield (lambda j=jj: phase1_j(ch, j))
            yield lambda: phase1_v(ch, 0)
            yield lambda: phase1_v(ch, 1)

        def normalize(pso, pb, hp, yt2):
            rv = p2r.tile([1, QC], F32, tag="rv")
            nc.vector.reciprocal(rv[:], pso[64:65, :])
            scr = drp.tile([QC], F32)
            nc.sync.dma_start(scr[None, :], rv[:])
            rbc = p2r.tile([HD, QC], F32, tag="rbc")
            nc.sync.dma_start(rbc[:], scr[None, :].to_broadcast((HD, QC)))
            nc.vector.tensor_mul(yt2[pb:pb + 64, hp, :], pso[0:64, :], rbc[:])

        def attn_qc0_head(h, yt2):
                pb, hp = 64 * (h % 2), h // 2
                pso = ps_o.tile([128, QC], F32, tag="pso")
                for p in range(2):
                    cs = 256 * p
                    pss = ps_s.tile([128, 2, QC], F32, tag="pss")
                    for pl in range(2):
                        kt = 2 * p + pl
                        nc.tensor.matmul(
                            pss[:, pl, cs:512],
                            kb[pb:pb + 64, hp, kt * 128:(kt + 1) * 128],
                            qb[pb:pb + 64, hp, cs:512],
                            start=True, stop=True)
                    pt = p2.tile([128, 2, QC], BF16, tag="pt0")
                    nc.scalar.activation(pt[:, :, cs:512], pss[:, :, cs:512],
                                         EXP, scale=SCALE)
                    for pl in range(2):
                        kt = 2 * p + pl
                        b2 = 128 * kt + 128
                        nc.gpsimd.affine_select(
                            out=pt[:, pl, cs:b2], in_=pt[:, pl, cs:b2],
                            compare_op=mybir.AluOpType.is_ge,
                            fill=0.0, base=cs - 128 * kt,
                            channel_multiplier=-1, pattern=[[1, b2 - cs]])
                        nc.tensor.matmul(
                            pso[0:65, cs:512],
                            vb[:, kt, h * 65:(h + 1) * 65],
                            pt[:, pl, cs:512],
                            start=(kt == 0), stop=(kt == 3),
                            skip_group_check=True)
                normalize(pso, pb, hp, yt2)

        def attn_qc_head(qc, h, yt2):
                pb, hp = 64 * (h % 2), h // 2
                pso = ps_o.tile([128, QC], F32, tag="pso")
                npair = 2 * qc + 2
                for p in range(npair):
                    jj0 = 2 * p - 4 * qc
                    cs = 128 * max(0, jj0)
                    pss = ps_s.tile([128, 2, QC], F32, tag="pss")
                    for pl in range(2):
                        kt = 2 * p + pl
                        nc.tensor.matmul(
                            pss[:, pl, cs:512],
                            khl[pb:pb + 64, hp, :, kt * 128:(kt + 1) * 128],
                            q8[pb:pb + 64, hp, bass.ds(qc * QC + cs, 512 - cs)]
                            [:, None, :].to_broadcast((64, 2, 512 - cs)),
                            start=True, stop=True, perf_mode=DR)
                    pt = p2.tile([128, 2, QC], F8, tag="pt")
                    nc.scalar.activation(pt[:, :, cs:512], pss[:, :, cs:512],
                                         EXP, scale=SCALE, bias=ebias[:])
                    for pl in range(2):
                        jj = jj0 + pl
                        if 0 <= jj <= 3:
                            a = cs
                            b2 = min(128 * jj + 128, 512)
                            nc.gpsimd.affine_select(
                                out=pt[:, pl, a:b2], in_=pt[:, pl, a:b2],
                                compare_op=mybir.AluOpType.is_ge,
                                fill=0.0,
                                base=qc * QC + a - 128 * (2 * p + pl),
                                channel_multiplier=-1, pattern=[[1, b2 - a]])
                    nc.tensor.matmul(
                        pso[0:128, cs:512],
                        v8[:, 2 * p:2 * p + 2, h, :],
                        pt[:, :, cs:512],
                        start=(p == 0), stop=(p == npair - 1),
                        perf_mode=DR, skip_group_check=True)
                normalize(pso, pb, hp, yt2)

        def proj_piece(qc, si, no, yt2):
            psp = ps_a.tile([128, 512], F32, tag="pa")
            for j in range(4):
                nc.tensor.matmul(
                    psp[:],
                    yt2[:, j, si * 128:(si + 1) * 128],
                    wp[:, j, no * 512:(no + 1) * 512],
                    start=(j == 0), stop=(j == 3))
            osb = p3.tile([128, 512], F32, tag="osb")
            nc.vector.tensor_copy(osb[:], psp[:])
            nc.sync.dma_start(
                out_d[bass.ds(qc * QC + si * 128, 128),
                      bass.ds(no * 512, 512)],
                osb[:])

        def proj_units(qc, yt2):
            for si in range(4):
                for no in range(2):
                    yield (lambda s=si, n=no: proj_piece(qc, s, n, yt2))

        # ---- schedule: software pipeline at head-pair granularity.
        # Attention stage qc runs one phase1-chunk-pair early so the ACT
        # engine (exp = bottleneck) always has S tiles queued.
        xh01 = p1x.tile([128, 8, 512], F8, tag="xh01")
        xl01 = p1x.tile([128, 8, 512], F8, tag="xl01")
        xhR = p1x.tile([128, 8, 1536], F8, tag="xhR")
        xlR = p1x.tile([128, 8, 1536], F8, tag="xlR")
        nc.sync.dma_start(xh01[:], xh_d.ap()[:, 0:512]
                          .rearrange("(cc p) f -> p cc f", p=128))
        nc.sync.dma_start(wqkh[:], r8(wqkh_d))
        nc.sync.dma_start(xl01[:], xl_d.ap()[:, 0:512]
                          .rearrange("(cc p) f -> p cc f", p=128))
        nc.sync.dma_start(wqkl[:], r8(wqkl_d))
        nc.sync.dma_start(wvh[:], r8(wvh_d))
        nc.sync.dma_start(wvl[:], r8(wvl_d))
        nc.sync.dma_start(xhR[:], xh_d.ap()[:, 512:2048]
                          .rearrange("(cc p) f -> p cc f", p=128))
        nc.sync.dma_start(xlR[:], xl_d.ap()[:, 512:2048]
                          .rearrange("(cc p) f -> p cc f", p=128))
        nc.sync.dma_start(wp[:], wp_d.ap())
        for ch in range(8):
            if ch < 2:
                xtiles[ch] = (xh01[:, :, ch * 256:(ch + 1) * 256],
                              xl01[:, :, ch * 256:(ch + 1) * 256])
            else:
                o = (ch - 2) * 256
                xtiles[ch] = (xhR[:, :, o:o + 256], xlR[:, :, o:o + 256])

        def jgrp(ca, cb, hp):
            phase1_j(ca, hp); phase1_j(ca, hp + 4)
            phase1_j(cb, hp); phase1_j(cb, hp + 4)

        def vgrp(ca, cb):
            phase1_v(ca, 0); phase1_v(ca, 1)
            phase1_v(cb, 0); phase1_v(cb, 1)

        jgrp(0, 1, 0)
        vgrp(0, 1)
        yt0 = p2y.tile([128, 4, QC], BF16, tag="yt2")
        yt1 = p2y.tile([128, 4, QC], BF16, tag="yt2")
        # stage A: qc0 heads + qc1 heads one hp behind; phase1 ch2/3
        for hp in range(4):
            if hp > 0:
                jgrp(0, 1, hp)
            attn_qc0_head(2 * hp, yt0)
            attn_qc0_head(2 * hp + 1, yt0)
            jgrp(2, 3, hp)
            if hp == 0:
                vgrp(2, 3)
            else:
                attn_qc_head(1, 2 * (hp - 1), yt1)
                attn_qc_head(1, 2 * hp - 1, yt1)
        attn_qc_head(1, 6, yt1)
        attn_qc_head(1, 7, yt1)

        yt2_ = p2y.tile([128, 4, QC], BF16, tag="yt2")
        # stage B: qc2 heads; phase1 ch4/5; proj0
        pj0 = list(proj_units(0, yt0))
        for hp in range(4):
            jgrp(4, 5, hp)
            if hp == 0:
                vgrp(4, 5)
            attn_qc_head(2, 2 * hp, yt2_)
            pj0[2 * hp]()
            attn_qc_head(2, 2 * hp + 1, yt2_)
            pj0[2 * hp + 1]()

        yt3 = p2y.tile([128, 4, QC], BF16, tag="yt2")
        # stage C: qc3 heads; phase1 ch6/7; proj1
        pj1 = list(proj_units(1, yt1))
        pj2 = list(proj_units(2, yt2_))
        for hp in range(4):
            jgrp(6, 7, hp)
            if hp == 0:
                vgrp(6, 7)
            attn_qc_head(3, 2 * hp, yt3)
            pj1[2 * hp](); pj2[2 * hp]()
            attn_qc_head(3, 2 * hp + 1, yt3)
            pj1[2 * hp + 1](); pj2[2 * hp + 1]()

        for u in proj_units(3, yt3):
            u()


_NC_CACHE = {}


def _get_nc():
    if "nc" not in _NC_CACHE:
        _NC_CACHE["nc"] = _build_nc()
    return _NC_CACHE["nc"]


def _hilo(a):
    hi = np.asarray(a, dtype=F8NP)
    lo = np.asarray(a - hi.astype(np.float32), dtype=F8NP)
    return np.ascontiguousarray(hi), np.ascontiguousarray(lo)


def _make_in_maps(x, w_attn, w_proj):
    in_maps = []
    for c in range(8):
        b, hh = c // 2, c % 2
        qs = 512 * hh
        wqk = np.concatenate(
            [w_attn[:, qs:qs + 512], w_attn[:, 1024 + qs:1024 + qs + 512]],
            axis=1)
        wqkh, wqkl = _hilo(SW * wqk)
        wvh, wvl = _hilo(SW * w_attn[:, 2048 + qs:2048 + qs + 512])
        xh, xl = _hilo(x[b].T)
        wp2 = np.ascontiguousarray(
            (w_proj[qs:qs + 512, :] / SW).reshape(4, 128, 1024)
            .transpose(1, 0, 2).astype(BFNP))
        in_maps.append({
            "xh": xh, "xl": xl,
            "wqkh": wqkh, "wqkl": wqkl,
            "wvh": wvh, "wvl": wvl,
            "wp": wp2,
        })
    return in_maps


def kernel(x, w_attn, w_proj):
    x = np.ascontiguousarray(np.asarray(x, dtype=np.float32))
    w_attn = np.ascontiguousarray(np.asarray(w_attn, dtype=np.float32))
    w_proj = np.ascontiguousarray(np.asarray(w_proj, dtype=np.float32))
    in_maps = _make_in_maps(x, w_attn, w_proj)

    nc = _get_nc()
    res = run_bass_kernel_spmd(nc, in_maps, list(range(8))).results

    out = np.empty((B, T, C), dtype=np.float32)
    for b in range(B):
        out[b] = res[2 * b]["out"] + res[2 * b + 1]["out"]
    return out
